# revision 3
# baseline (speedup 1.0000x reference)
"""Trainium2 Bass kernel for CausalSelfAttention (QAT fake-quant + low-rank
adapters + RMSNorm + partial RoPE + GQA causal attention).

Sharding: 8 cores = 2 (batch) x 4 (kv-head groups). Core c handles batch
b = c // 4 and kv group g = c % 4: q heads 4g..4g+3, kv head g. Each core
computes a partial out-projection (its y-column slice x Wproj column slice);
the host sums the 4 bf16 partials per batch element in f32.

Weight preparation happens on the host: the per-row int8 fake-quant is
computed exactly in f32 (IEEE divide + round-half-even, matching jax-on-CPU),
then re-scaled and FOLDED with the low-rank adapter product A@B into a single
effective weight matrix per projection, shipped in bf16. On device the whole
module is then: 3 plain matmuls + RMSNorm + RoPE + causal attention + 1 plain
matmul.

Device schedule: a single software-pipelined pass over 4 token macros of 512.
Within macro m the emission interleaves, at matmul granularity:
  - projections for macro m (x @ Wqkv) + norm/rope/transpose closures,
  - attention for macro m-1 (scores -> exp -> causal mask -> P@V),
  - the out-projection of macro m-2.
Softmax denominators are accumulated on the vector engine (bf16 adds over the
exp tiles) and partition-reduced/broadcast with a single ones-matmul per
(head, i-half), instead of one ones-matmul per exp tile. Scores are built
transposed ([j, i]) so P.T is never materialized; the diagonal blocks are
computed with restricted i-ranges so only ~causal work is done.

PSUM (8 banks) is managed manually: pq pool (2), pkv halves (1), transposes
(1), score slots (2), P@V halves (1), out-proj halves (1).
"""

import sys

sys.path.insert(0, '/opt/trn_rl_repo')

from contextlib import ExitStack

import numpy as np

import concourse.bass as bass
import concourse.bacc as bacc
import concourse.tile as tile
from concourse import mybir
from concourse.bass_utils import run_bass_kernel_spmd
from concourse.masks import make_identity

F32 = mybir.dt.float32
BF16 = mybir.dt.bfloat16
AF = mybir.ActivationFunctionType
ALU = mybir.AluOpType

B, S, DIM = 2, 2048, 2048
NH, NKV = 16, 4
HD = 128
RANK = 16
ROPE_DIMS = 64
HALF = ROPE_DIMS // 2  # 32
RBASE = 10000.0
EPS = 1.1920929e-7
EPS128 = 128.0 * EPS
SQRT_HD = float(np.sqrt(128.0))

NT = S // 128            # 16 token tiles of 128
NM = S // 512            # 4 token macros of 512
ND = DIM // 128          # 16 contraction chunks
QF = 4 * HD              # 512 q features per core
KF = HD                  # 128 kv features per core
IW = 256                 # attention i-tile width (half macro)


def _headbc(ap, nheads):
    """View a [128, 32] AP as [128, nheads, 32] with zero head stride."""
    return bass.AP(tensor=ap.tensor, offset=ap.offset,
                   ap=[list(ap.ap[0]), [0, nheads], list(ap.ap[1])])


def build_program():
    nc = bacc.Bacc(None, target_bir_lowering=False)

    xT = nc.declare_dram_parameter("xT", [DIM, S], BF16, isOutput=False)
    wq = nc.declare_dram_parameter("wq", [DIM, QF], BF16, isOutput=False)
    wkv = nc.declare_dram_parameter("wkv", [DIM, 2 * KF], BF16, isOutput=False)
    wp = nc.declare_dram_parameter("wp", [QF, DIM], BF16, isOutput=False)
    cs = nc.declare_dram_parameter("cs", [S, HALF], F32, isOutput=False)
    sn = nc.declare_dram_parameter("sn", [S, HALF], F32, isOutput=False)
    gn = nc.declare_dram_parameter("gn", [1, 4], F32, isOutput=False)
    outT = nc.declare_dram_parameter("outT", [DIM, S], BF16, isOutput=True)

    with tile.TileContext(nc) as tc:
        st = ExitStack()
        const = st.enter_context(tc.tile_pool(name="const", bufs=1))

        ident = const.tile([128, 128], BF16)
        make_identity(nc, ident)
        ones_t = const.tile([128, 128], BF16)
        nc.vector.memset(ones_t, 1.0)
        eps_t = const.tile([128, 1], F32)
        nc.vector.memset(eps_t, EPS128)
        gmul = const.tile([128, 5], F32)
        cos_t = const.tile([128, NT, HALF], F32)
        sin_t = const.tile([128, NT, HALF], F32)

        wq_i = const.tile([128, ND, QF], BF16)
        wkv_i = const.tile([128, ND, 2 * KF], BF16)
        wp_i = const.tile([128, QF // 128, DIM], BF16)

        qT = const.tile([128, 4, S], BF16)    # [hd, head, token]
        kT = const.tile([128, S], BF16)
        vres = const.tile([128, NT, HD], BF16)
        yT = const.tile([128, 4, S], BF16)

        xpool = st.enter_context(tc.tile_pool(name="xp", bufs=2 * ND))
        clp = st.enter_context(tc.tile_pool(name="clp", bufs=2))
        epool = st.enter_context(tc.tile_pool(name="ep", bufs=52))
        accp = st.enter_context(tc.tile_pool(name="accp", bufs=4))
        invp = st.enter_context(tc.tile_pool(name="invp", bufs=3))
        osbp = st.enter_context(tc.tile_pool(name="osbp", bufs=3))

        ps_q = st.enter_context(tc.tile_pool(name="psq", bufs=2, space="PSUM"))
        pman = st.enter_context(tc.tile_pool(name="pman", bufs=1, space="PSUM"))
        pkv_t = pman.tile([128, 2, 2 * KF], F32)    # 1 bank, 2 halves
        tp_t = pman.tile([128, 640], BF16)          # 1 bank (1.25KB)
        psc_t = pman.tile([128, 4, IW], F32)        # 2 banks, 4 score slots
        pyd_t = pman.tile([128, 2, IW], F32)        # 1 bank, 2 P@V halves
        po_t = pman.tile([128, 2, IW], F32)         # 1 bank, 2 outproj halves

        xf_tiles = {}
        counters = {"psc": 0, "po": 0}
        unit_data = {}

        # ------------------------------------------------------------------
        def xdma_stream(m):
            for d in range(ND):
                xf = xpool.tile([128, 512], BF16, tag="xf", name="xf")
                nc.sync.dma_start(out=xf, in_=xT[d * 128:(d + 1) * 128,
                                                m * 512:(m + 1) * 512])
                xf_tiles[(m, d)] = xf
                yield 60

        # ------------------------------------------------------------------
        def closure(tt, pq, kv):
            """norm + rope + transpose for token tile tt (q in pq psum,
            k|v in kv psum half)."""
            stats = clp.tile([128, 5], F32, tag="stats", name="stats")
            sqscr = clp.tile([128, 128], F32, tag="sqscr", name="sqscr")
            for c in range(4):
                nc.scalar.activation(out=sqscr, in_=pq[:, c * 128:(c + 1) * 128],
                                     func=AF.Square, accum_out=stats[:, c:c + 1])
            nc.scalar.activation(out=sqscr, in_=kv[:, 0:KF],
                                 func=AF.Square, accum_out=stats[:, 4:5])
            nc.scalar.activation(out=vres[:, tt, :], in_=kv[:, KF:2 * KF],
                                 func=AF.Copy)
            yield 150
            nc.scalar.activation(out=stats, in_=stats, func=AF.Sqrt, bias=eps_t)
            nc.vector.reciprocal_approx_fast(out=stats, in_=stats)
            nc.vector.tensor_mul(stats, stats, gmul)
            yield 100
            # rope q (in place, in psum)
            q4 = pq.rearrange("p (h c) -> p h c", h=4)
            x1, x2 = q4[:, :, 0:HALF], q4[:, :, HALF:ROPE_DIMS]
            cb4 = _headbc(cos_t[:, tt, :], 4)
            sb4 = _headbc(sin_t[:, tt, :], 4)
            t1 = clp.tile([128, 4, HALF], F32, tag="t1", name="t1")
            t2 = clp.tile([128, 4, HALF], F32, tag="t2", name="t2")
            t3 = clp.tile([128, 4, HALF], F32, tag="t3", name="t3")
            t4 = clp.tile([128, 4, HALF], F32, tag="t4", name="t4")
            nc.vector.tensor_mul(t1, x1, cb4)
            nc.vector.tensor_mul(t2, x2, sb4)
            nc.vector.tensor_mul(t3, x2, cb4)
            nc.vector.tensor_mul(t4, x1, sb4)
            nc.vector.tensor_add(x1, t1, t2)
            nc.vector.tensor_sub(x2, t3, t4)
            yield 250
            # rope k
            k1, k2 = kv[:, 0:HALF], kv[:, HALF:ROPE_DIMS]
            u1 = clp.tile([128, HALF], F32, tag="u1", name="u1")
            u2 = clp.tile([128, HALF], F32, tag="u2", name="u2")
            u3 = clp.tile([128, HALF], F32, tag="u3", name="u3")
            u4 = clp.tile([128, HALF], F32, tag="u4", name="u4")
            nc.vector.tensor_mul(u1, k1, cos_t[:, tt, :])
            nc.vector.tensor_mul(u2, k2, sin_t[:, tt, :])
            nc.vector.tensor_mul(u3, k2, cos_t[:, tt, :])
            nc.vector.tensor_mul(u4, k1, sin_t[:, tt, :])
            nc.vector.tensor_add(k1, u1, u2)
            nc.vector.tensor_sub(k2, u3, u4)
            yield 200
            # per-head normalization -> bf16
            qkf = clp.tile([128, 640], BF16, tag="qkf", name="qkf")
            for c in range(4):
                nc.vector.tensor_scalar(
                    out=qkf[:, c * 128:(c + 1) * 128],
                    in0=pq[:, c * 128:(c + 1) * 128],
                    scalar1=stats[:, c:c + 1], scalar2=None, op0=ALU.mult)
            nc.vector.tensor_scalar(
                out=qkf[:, 512:640], in0=kv[:, 0:KF],
                scalar1=stats[:, 4:5], scalar2=None, op0=ALU.mult)
            yield 400
            # transpose to [hd, token]
            for c in range(5):
                nc.tensor.matmul(tp_t[:, c * 128:(c + 1) * 128],
                                 qkf[:, c * 128:(c + 1) * 128], ident,
                                 is_transpose=True, start=True, stop=True,
                                 skip_group_check=True)
            yield 300
            tsl = slice(tt * 128, (tt + 1) * 128)
            nc.scalar.activation(
                out=qT[:, :, tsl],
                in_=tp_t[:, 0:512].rearrange("p (c t) -> p c t", c=4),
                func=AF.Copy)
            nc.scalar.activation(out=kT[:, tsl], in_=tp_t[:, 512:640],
                                 func=AF.Copy)
            yield 200

        # ------------------------------------------------------------------
        def proj_stream(m):
            for tsub in range(4):
                tt = 4 * m + tsub
                tsl = slice(tsub * 128, (tsub + 1) * 128)
                pq = ps_q.tile([128, QF], F32, name="pq", tag="pq")
                kv = pkv_t[:, tt % 2, :]
                for d in range(ND):
                    lhs = xf_tiles[(m, d)][:, tsl]
                    nc.tensor.matmul(pq, lhs, wq_i[:, d, :],
                                     start=(d == 0), stop=(d == ND - 1))
                    nc.tensor.matmul(kv, lhs, wkv_i[:, d, :],
                                     start=(d == 0), stop=(d == ND - 1),
                                     skip_group_check=True)
                    yield 320
                yield from closure(tt, pq, kv)

        # ------------------------------------------------------------------
        def sc_unit(am, h, u):
            """scores + exp + mask + denominator accumulation for
            (head h, i-half u) of attention macro am."""
            i0 = am * 512 + u * IW
            jd = 4 * am + 2 * u
            jlist = [(jd, 0, IW), (jd + 1, 128, 128)] + \
                    [(jc, 0, IW) for jc in range(jd)]
            es = []
            acc = accp.tile([128, IW], BF16, tag="acc", name="acc")
            for idx, (jc, off, w) in enumerate(jlist):
                pscore = psc_t[:, counters["psc"] % 4, 0:w]
                counters["psc"] += 1
                nc.tensor.matmul(pscore, kT[:, jc * 128:(jc + 1) * 128],
                                 qT[:, h, i0 + off:i0 + IW],
                                 start=True, stop=True, skip_group_check=True)
                e = epool.tile([128, w], BF16, tag="e", name="e",
                               padded_shape=[128, IW])
                nc.scalar.activation(out=e, in_=pscore, func=AF.Exp)
                if idx < 2:
                    # causal: zero entries with j > i in the diagonal block
                    nc.gpsimd.affine_select(
                        out=e[:, 0:128], in_=e[:, 0:128],
                        compare_op=ALU.is_ge, fill=0.0,
                        base=0, channel_multiplier=-1, pattern=[[1, 128]])
                if idx == 0:
                    nc.vector.tensor_copy(out=acc, in_=e)
                else:
                    nc.vector.tensor_add(acc[:, off:off + w],
                                         acc[:, off:off + w], e)
                es.append((jc, off, w, e))
                yield 215
            unit_data[(am, h, u)] = (es, acc)

        def av_unit(am, h, u):
            i0 = am * 512 + u * IW
            es, acc = unit_data.pop((am, h, u))
            py = pyd_t[:, u, :]
            for idx, (jc, off, w, e) in enumerate(es):
                nc.tensor.matmul(py[:, off:off + w], vres[:, jc, :], e,
                                 start=(idx == 0), stop=(idx == len(es) - 1),
                                 skip_group_check=True)
                yield 110
            pd = psc_t[:, counters["psc"] % 4, :]
            counters["psc"] += 1
            nc.tensor.matmul(pd, ones_t, acc, start=True, stop=True,
                             skip_group_check=True)
            inv = invp.tile([128, IW], F32, tag="inv", name="inv")
            nc.vector.reciprocal_approx_fast(out=inv, in_=pd)
            nc.vector.tensor_mul(yT[:, h, i0:i0 + IW], py, inv)
            yield 300

        def attn_stream(am):
            units = [(h, u) for h in range(4) for u in range(2)]
            pend = []
            for i, (h, u) in enumerate(units):
                yield from sc_unit(am, h, u)
                pend.append((h, u))
                if i >= 2:
                    hh, uu = pend.pop(0)
                    yield from av_unit(am, hh, uu)
            while pend:
                hh, uu = pend.pop(0)
                yield from av_unit(am, hh, uu)

        # ------------------------------------------------------------------
        def op_stream(m2):
            for oc in range(ND):
                osl = slice(oc * 128, (oc + 1) * 128)
                osb = osbp.tile([128, 512], BF16, tag="osb", name="osb")
                for u in range(2):
                    po = po_t[:, counters["po"] % 2, :]
                    counters["po"] += 1
                    isl = slice(m2 * 512 + u * IW, m2 * 512 + (u + 1) * IW)
                    for fc in range(4):
                        nc.tensor.matmul(po, wp_i[:, fc, osl], yT[:, fc, isl],
                                         start=(fc == 0), stop=(fc == 3),
                                         skip_group_check=True)
                    nc.vector.tensor_copy(out=osb[:, u * IW:(u + 1) * IW],
                                          in_=po)
                    yield 480
                nc.sync.dma_start(out=outT[osl, m2 * 512:(m2 + 1) * 512],
                                  in_=osb)
                yield 20

        # ------------------------------------------------------------------
        def merge(streams):
            # lockstep merge: always advance the stream with the least
            # emitted PE-cost, so no engine queue runs far ahead of the rest
            prog = [0.0] * len(streams)
            live = list(range(len(streams)))
            gens = [iter(s) for s in streams]
            while live:
                i = min(live, key=lambda j: prog[j])
                try:
                    prog[i] += next(gens[i])
                except StopIteration:
                    live.remove(i)

        # ------------------------------------------------------------------
        # preload: weights + macro-0 x + small constants
        for d in range(ND):
            nc.sync.dma_start(out=wq_i[:, d, :],
                              in_=wq[d * 128:(d + 1) * 128, :])
            nc.sync.dma_start(out=wkv_i[:, d, :],
                              in_=wkv[d * 128:(d + 1) * 128, :])
            xf = xpool.tile([128, 512], BF16, tag="xf", name="xf")
            nc.sync.dma_start(out=xf, in_=xT[d * 128:(d + 1) * 128, 0:512])
            xf_tiles[(0, d)] = xf
            if d < 4:
                nc.scalar.dma_start(out=wp_i[:, d, :],
                                    in_=wp[d * 128:(d + 1) * 128, :])
        nc.sync.dma_start(
            out=cos_t, in_=cs[:, :].rearrange("(a p) d -> p a d", p=128))
        nc.sync.dma_start(
            out=sin_t, in_=sn[:, :].rearrange("(a p) d -> p a d", p=128))
        nc.sync.dma_start(out=gmul[:, 0:4], in_=gn[:, :].to_broadcast([128, 4]))
        nc.vector.memset(gmul[:, 4:5], SQRT_HD)

        for m in range(NM):
            streams = [proj_stream(m)]
            if m + 1 < NM:
                streams.append(xdma_stream(m + 1))
            if m >= 1:
                streams.append(attn_stream(m - 1))
            if m >= 2:
                streams.append(op_stream(m - 2))
            merge(streams)
        merge([attn_stream(NM - 1), op_stream(NM - 2)])
        merge([op_stream(NM - 1)])

        st.close()

    nc.finalize()
    return nc


# ======================================================================
# host side


def _fold(W, A, Bm, bf16):
    """fq(W).T + A @ B as a single bf16 matrix [in_dim, out_dim]."""
    f32 = np.float32
    W = np.asarray(W, f32)
    s = np.maximum(np.max(np.abs(W), axis=1) / f32(127.0),
                   f32(1.0 / 127.0)).astype(f32)
    Wint = np.round(W / s[:, None]).astype(f32)
    eff = (Wint * s[:, None]).T + np.asarray(A, f32) @ np.asarray(Bm, f32)
    return eff.astype(bf16)


def make_in_maps(x, Wq, Wk, Wv, Wproj, q_gain, q_A, q_B, k_A, k_B, v_A, v_B,
                 proj_A, proj_B):
    import ml_dtypes
    bf16 = ml_dtypes.bfloat16
    f32 = np.float32

    wq_eff = _fold(Wq, q_A, q_B, bf16)          # [2048, 2048]
    wk_eff = _fold(Wk, k_A, k_B, bf16)          # [2048, 512]
    wv_eff = _fold(Wv, v_A, v_B, bf16)          # [2048, 512]
    wp_eff = _fold(Wproj, proj_A, proj_B, bf16)  # [2048, 2048]

    inv_freq = (f32(1.0) / (f32(RBASE) ** (np.arange(0, ROPE_DIMS, 2,
                dtype=f32) / f32(ROPE_DIMS)))).astype(f32)
    t = np.arange(S, dtype=f32)
    freqs = np.outer(t, inv_freq).astype(f32)
    cos = np.cos(freqs).astype(f32)
    sin = np.sin(freqs).astype(f32)
    q_gain = np.asarray(q_gain, f32)
    x = np.asarray(x, f32)

    in_maps = []
    for c in range(8):
        b, g = divmod(c, 4)
        fq0, fq1 = 512 * g, 512 * (g + 1)
        fk0, fk1 = 128 * g, 128 * (g + 1)
        in_maps.append({
            "xT": np.ascontiguousarray(x[b].T).astype(bf16),
            "wq": np.ascontiguousarray(wq_eff[:, fq0:fq1]),
            "wkv": np.ascontiguousarray(
                np.concatenate([wk_eff[:, fk0:fk1], wv_eff[:, fk0:fk1]],
                               axis=1)),
            "wp": np.ascontiguousarray(wp_eff[fq0:fq1, :]),
            "cs": cos,
            "sn": sin,
            "gn": np.ascontiguousarray(q_gain[None, 4 * g:4 * (g + 1)]),
        })
    return in_maps


def assemble(res):
    out = np.empty((B, S, DIM), np.float32)
    for b in range(B):
        acc = res.results[4 * b]["outT"].astype(np.float32)
        for g in range(1, 4):
            acc = acc + res.results[4 * b + g]["outT"].astype(np.float32)
        out[b] = acc.T
    return out


_PROGRAM = None


def kernel(**inputs):
    global _PROGRAM
    if _PROGRAM is None:
        _PROGRAM = build_program()
    in_maps = make_in_maps(**inputs)
    res = run_bass_kernel_spmd(_PROGRAM, in_maps, core_ids=list(range(8)))
    return assemble(res)


# revision 4
# speedup vs baseline: 1.7085x; 1.7085x over previous
"""Trainium2 Bass kernel for CausalSelfAttention (QAT fake-quant + low-rank
adapters + RMSNorm + partial RoPE + GQA causal attention).

Sharding: 8 cores = 2 (batch) x 4 (kv-head groups). Core c handles batch
b = c // 4 and kv group g = c % 4: q heads 4g..4g+3, kv head g. Each core
computes a partial out-projection (its y-column slice x Wproj column slice);
the host sums the 4 bf16 partials per batch element in f32.

Weight preparation happens on the host: the per-row int8 fake-quant is
computed exactly in f32 (IEEE divide + round-half-even, matching jax-on-CPU),
then re-scaled and FOLDED with the low-rank adapter product A@B into a single
effective weight matrix per projection, shipped in bf16. On device the whole
module is then: 2 plain matmul passes (q|kv), RMSNorm + RoPE + transpose,
causal attention, and 1 plain out-projection matmul pass.

Phase B (projections): per 128-token tile, x-chunks stationary, fused weight
blocks moving; norm stats / RoPE run in-PSUM, one bf16 round at the per-head
normalization, then PE transposes to [hd, token] layout.

Phase C (attention + out-proj): scores are built transposed ([j, i]) so P.T
is never materialized; softmax denominators are accumulated on the vector
engine (bf16 adds over exp tiles) and partition-reduced/broadcast with one
ones-matmul per (head, macro). Diagonal blocks use restricted i-ranges so
only ~causal work is done. The out-projection of macro m-1 is interleaved at
matmul granularity into macro m's attention to keep the PE busy while exps
drain.
"""

import sys

sys.path.insert(0, '/opt/trn_rl_repo')

from contextlib import ExitStack

import numpy as np

import concourse.bass as bass
import concourse.bacc as bacc
import concourse.tile as tile
from concourse import mybir
from concourse.bass_utils import run_bass_kernel_spmd
from concourse.masks import make_identity

F32 = mybir.dt.float32
BF16 = mybir.dt.bfloat16
AF = mybir.ActivationFunctionType
ALU = mybir.AluOpType

B, S, DIM = 2, 2048, 2048
NH, NKV = 16, 4
HD = 128
RANK = 16
ROPE_DIMS = 64
HALF = ROPE_DIMS // 2  # 32
RBASE = 10000.0
EPS = 1.1920929e-7
EPS128 = 128.0 * EPS
SQRT_HD = float(np.sqrt(128.0))

NT = S // 128            # 16 token tiles of 128
NM = S // 512            # 4 token macros of 512
ND = DIM // 128          # 16 contraction chunks
QF = 4 * HD              # 512 q features per core
KF = HD                  # 128 kv features per core


def _headbc(ap, nheads):
    """View a [128, 32] AP as [128, nheads, 32] with zero head stride."""
    return bass.AP(tensor=ap.tensor, offset=ap.offset,
                   ap=[list(ap.ap[0]), [0, nheads], list(ap.ap[1])])


def build_program():
    nc = bacc.Bacc(None, target_bir_lowering=False)

    xT = nc.declare_dram_parameter("xT", [DIM, S], BF16, isOutput=False)
    wq = nc.declare_dram_parameter("wq", [DIM, QF], BF16, isOutput=False)
    wkv = nc.declare_dram_parameter("wkv", [DIM, 2 * KF], BF16, isOutput=False)
    wp = nc.declare_dram_parameter("wp", [QF, DIM], BF16, isOutput=False)
    cs = nc.declare_dram_parameter("cs", [S, HALF], F32, isOutput=False)
    sn = nc.declare_dram_parameter("sn", [S, HALF], F32, isOutput=False)
    gn = nc.declare_dram_parameter("gn", [1, 4], F32, isOutput=False)
    outT = nc.declare_dram_parameter("outT", [DIM, S], BF16, isOutput=True)

    with tile.TileContext(nc) as tc:
        st = ExitStack()
        const = st.enter_context(tc.tile_pool(name="const", bufs=1))

        ident = const.tile([128, 128], BF16)
        make_identity(nc, ident)
        ones_t = const.tile([128, 128], BF16)
        nc.vector.memset(ones_t, 1.0)
        eps_t = const.tile([128, 1], F32)
        nc.vector.memset(eps_t, EPS128)
        gmul = const.tile([128, 5], F32)
        cos_t = const.tile([128, NT, HALF], F32)
        sin_t = const.tile([128, NT, HALF], F32)

        wq_i = const.tile([128, ND, QF], BF16)
        wkv_i = const.tile([128, ND, 2 * KF], BF16)
        wp_i = const.tile([128, QF // 128, DIM], BF16)

        qT = const.tile([128, 4, S], BF16)    # [hd, head, token]
        kT = const.tile([128, S], BF16)
        vres = const.tile([128, NT, HD], BF16)
        yT = const.tile([128, 4, S], BF16)

        xpool = st.enter_context(tc.tile_pool(name="xp", bufs=2 * ND))
        clp = st.enter_context(tc.tile_pool(name="clp", bufs=2))
        epool = st.enter_context(tc.tile_pool(name="ep", bufs=40))
        accp = st.enter_context(tc.tile_pool(name="accp", bufs=3))
        invp = st.enter_context(tc.tile_pool(name="invp", bufs=3))
        osbp = st.enter_context(tc.tile_pool(name="osbp", bufs=3))

        xf_tiles = {}

        def xdma(m):
            for d in range(ND):
                xf = xpool.tile([128, 512], BF16, tag="xf", name="xf")
                nc.sync.dma_start(out=xf, in_=xT[d * 128:(d + 1) * 128,
                                                m * 512:(m + 1) * 512])
                xf_tiles[(m, d)] = xf

        # ------------------------------------------------------------------
        # phase B: projections + norm + rope + transpose
        bst = ExitStack()
        ps_q = bst.enter_context(tc.tile_pool(name="psq", bufs=3, space="PSUM"))
        ps_kv = bst.enter_context(tc.tile_pool(name="pskv", bufs=2, space="PSUM"))
        ps_tp = bst.enter_context(tc.tile_pool(name="pstp", bufs=2, space="PSUM"))

        def closure(tt, pq, kv):
            stats = clp.tile([128, 5], F32, tag="stats", name="stats")
            sqscr = clp.tile([128, 128], F32, tag="sqscr", name="sqscr")
            for c in range(4):
                nc.scalar.activation(out=sqscr, in_=pq[:, c * 128:(c + 1) * 128],
                                     func=AF.Square, accum_out=stats[:, c:c + 1])
            nc.scalar.activation(out=sqscr, in_=kv[:, 0:KF],
                                 func=AF.Square, accum_out=stats[:, 4:5])
            nc.scalar.activation(out=vres[:, tt, :], in_=kv[:, KF:2 * KF],
                                 func=AF.Copy)
            nc.scalar.activation(out=stats, in_=stats, func=AF.Sqrt, bias=eps_t)
            nc.vector.reciprocal_approx_fast(out=stats, in_=stats)
            nc.vector.tensor_mul(stats, stats, gmul)
            # rope q (in place, in psum)
            q4 = pq.rearrange("p (h c) -> p h c", h=4)
            x1, x2 = q4[:, :, 0:HALF], q4[:, :, HALF:ROPE_DIMS]
            cb4 = _headbc(cos_t[:, tt, :], 4)
            sb4 = _headbc(sin_t[:, tt, :], 4)
            t1 = clp.tile([128, 4, HALF], F32, tag="t1", name="t1")
            t2 = clp.tile([128, 4, HALF], F32, tag="t2", name="t2")
            t3 = clp.tile([128, 4, HALF], F32, tag="t3", name="t3")
            t4 = clp.tile([128, 4, HALF], F32, tag="t4", name="t4")
            nc.vector.tensor_mul(t1, x1, cb4)
            nc.vector.tensor_mul(t2, x2, sb4)
            nc.vector.tensor_mul(t3, x2, cb4)
            nc.vector.tensor_mul(t4, x1, sb4)
            nc.vector.tensor_add(x1, t1, t2)
            nc.vector.tensor_sub(x2, t3, t4)
            # rope k
            k1, k2 = kv[:, 0:HALF], kv[:, HALF:ROPE_DIMS]
            u1 = clp.tile([128, HALF], F32, tag="u1", name="u1")
            u2 = clp.tile([128, HALF], F32, tag="u2", name="u2")
            u3 = clp.tile([128, HALF], F32, tag="u3", name="u3")
            u4 = clp.tile([128, HALF], F32, tag="u4", name="u4")
            nc.vector.tensor_mul(u1, k1, cos_t[:, tt, :])
            nc.vector.tensor_mul(u2, k2, sin_t[:, tt, :])
            nc.vector.tensor_mul(u3, k2, cos_t[:, tt, :])
            nc.vector.tensor_mul(u4, k1, sin_t[:, tt, :])
            nc.vector.tensor_add(k1, u1, u2)
            nc.vector.tensor_sub(k2, u3, u4)
            # per-head normalization -> bf16
            qkf = clp.tile([128, 640], BF16, tag="qkf", name="qkf")
            for c in range(4):
                nc.vector.tensor_scalar(
                    out=qkf[:, c * 128:(c + 1) * 128],
                    in0=pq[:, c * 128:(c + 1) * 128],
                    scalar1=stats[:, c:c + 1], scalar2=None, op0=ALU.mult)
            nc.vector.tensor_scalar(
                out=qkf[:, 512:640], in0=kv[:, 0:KF],
                scalar1=stats[:, 4:5], scalar2=None, op0=ALU.mult)
            # transpose to [hd, token]
            tp = ps_tp.tile([128, 640], BF16, tag="tp", name="tp")
            for c in range(5):
                nc.tensor.matmul(tp[:, c * 128:(c + 1) * 128],
                                 qkf[:, c * 128:(c + 1) * 128], ident,
                                 is_transpose=True, start=True, stop=True,
                                 skip_group_check=True)
            tsl = slice(tt * 128, (tt + 1) * 128)
            nc.scalar.activation(
                out=qT[:, :, tsl],
                in_=tp[:, 0:512].rearrange("p (c t) -> p c t", c=4),
                func=AF.Copy)
            nc.scalar.activation(out=kT[:, tsl], in_=tp[:, 512:640],
                                 func=AF.Copy)

        # preload: weights + macro-0 x + small constants
        for d in range(ND):
            nc.sync.dma_start(out=wq_i[:, d, :],
                              in_=wq[d * 128:(d + 1) * 128, :])
            nc.sync.dma_start(out=wkv_i[:, d, :],
                              in_=wkv[d * 128:(d + 1) * 128, :])
            xf = xpool.tile([128, 512], BF16, tag="xf", name="xf")
            nc.sync.dma_start(out=xf, in_=xT[d * 128:(d + 1) * 128, 0:512])
            xf_tiles[(0, d)] = xf
            if d < 4:
                nc.scalar.dma_start(out=wp_i[:, d, :],
                                    in_=wp[d * 128:(d + 1) * 128, :])
        nc.sync.dma_start(
            out=cos_t, in_=cs[:, :].rearrange("(a p) d -> p a d", p=128))
        nc.sync.dma_start(
            out=sin_t, in_=sn[:, :].rearrange("(a p) d -> p a d", p=128))
        nc.sync.dma_start(out=gmul[:, 0:4], in_=gn[:, :].to_broadcast([128, 4]))
        nc.vector.memset(gmul[:, 4:5], SQRT_HD)

        pending = None
        for m in range(NM):
            for tsub in range(4):
                if tsub == 0 and m + 1 < NM:
                    xdma(m + 1)
                tt = 4 * m + tsub
                tsl = slice(tsub * 128, (tsub + 1) * 128)
                pq = ps_q.tile([128, QF], F32, name="pq", tag="pq")
                kv = ps_kv.tile([128, 2 * KF], F32, name="pkv", tag="pkv")
                for d in range(ND):
                    lhs = xf_tiles[(m, d)][:, tsl]
                    nc.tensor.matmul(pq, lhs, wq_i[:, d, :],
                                     start=(d == 0), stop=(d == ND - 1))
                    nc.tensor.matmul(kv, lhs, wkv_i[:, d, :],
                                     start=(d == 0), stop=(d == ND - 1))
                # defer the closure by one tile so psum fills while the
                # previous tile's closure runs
                if pending is not None:
                    closure(*pending)
                pending = (tt, pq, kv)
        closure(*pending)
        bst.close()

        # ------------------------------------------------------------------
        # phase C: attention + out-projection, interleaved at MM granularity
        cst = ExitStack()
        ps_s = cst.enter_context(tc.tile_pool(name="pss", bufs=4, space="PSUM"))
        ps_y = cst.enter_context(tc.tile_pool(name="psy", bufs=2, space="PSUM"))
        ps_o = cst.enter_context(tc.tile_pool(name="pso", bufs=2, space="PSUM"))

        unit_data = {}

        def sc_unit(am, h):
            i0 = am * 512
            jlist = [(4 * am + r, r * 128, 512 - r * 128) for r in range(4)]
            jlist += [(jc, 0, 512) for jc in range(4 * am)]
            es = []
            acc = accp.tile([128, 512], BF16, tag="acc", name="acc")
            for idx, (jc, off, w) in enumerate(jlist):
                pscore = ps_s.tile([128, 512], F32, tag="ps", name="ps")[:, 0:w]
                nc.tensor.matmul(pscore, kT[:, jc * 128:(jc + 1) * 128],
                                 qT[:, h, i0 + off:i0 + 512],
                                 start=True, stop=True)
                e = epool.tile([128, w], BF16, tag="e", name="e",
                               padded_shape=[128, 512])
                nc.scalar.activation(out=e, in_=pscore, func=AF.Exp)
                if idx < 4:
                    # causal: zero entries with j > i in the diagonal block
                    nc.gpsimd.affine_select(
                        out=e[:, 0:128], in_=e[:, 0:128],
                        compare_op=ALU.is_ge, fill=0.0,
                        base=0, channel_multiplier=-1, pattern=[[1, 128]])
                if idx == 0:
                    nc.vector.tensor_copy(out=acc, in_=e)
                else:
                    nc.vector.tensor_add(acc[:, off:off + w],
                                         acc[:, off:off + w], e)
                es.append((jc, off, w, e))
                yield 250
            unit_data[(am, h)] = (es, acc)

        def av_unit(am, h):
            i0 = am * 512
            es, acc = unit_data.pop((am, h))
            py = ps_y.tile([128, 512], F32, tag="py", name="py")
            for idx, (jc, off, w, e) in enumerate(es):
                nc.tensor.matmul(py[:, off:off + w], vres[:, jc, :], e,
                                 start=(idx == 0), stop=(idx == len(es) - 1),
                                 skip_group_check=True)
                yield 140
            pd = ps_s.tile([128, 512], F32, tag="ps", name="pd")
            nc.tensor.matmul(pd, ones_t, acc, start=True, stop=True)
            inv = invp.tile([128, 512], F32, tag="inv", name="inv")
            nc.vector.reciprocal_approx_fast(out=inv, in_=pd)
            nc.vector.tensor_mul(yT[:, h, i0:i0 + 512], py, inv)
            yield 300

        def attn_stream(am):
            pend = []
            for h in range(4):
                yield from sc_unit(am, h)
                pend.append(h)
                if h >= 2:
                    yield from av_unit(am, pend.pop(0))
            while pend:
                yield from av_unit(am, pend.pop(0))

        def op_stream(m2):
            isl = slice(m2 * 512, (m2 + 1) * 512)
            for oc in range(ND):
                osl = slice(oc * 128, (oc + 1) * 128)
                po = ps_o.tile([128, 512], F32, tag="po", name="po")
                for fc in range(4):
                    nc.tensor.matmul(po, wp_i[:, fc, osl], yT[:, fc, isl],
                                     start=(fc == 0), stop=(fc == 3))
                    yield 245
                osb = osbp.tile([128, 512], BF16, tag="osb", name="osb")
                nc.vector.tensor_copy(out=osb, in_=po)
                nc.sync.dma_start(out=outT[osl, isl], in_=osb)
                yield 30

        def merge(streams):
            prog = [0.0] * len(streams)
            live = list(range(len(streams)))
            gens = [iter(s) for s in streams]
            while live:
                i = min(live, key=lambda j: prog[j])
                try:
                    prog[i] += next(gens[i])
                except StopIteration:
                    live.remove(i)

        for am in range(NM):
            streams = [attn_stream(am)]
            if am >= 1:
                streams.append(op_stream(am - 1))
            merge(streams)
        merge([op_stream(NM - 1)])
        cst.close()

        st.close()

    nc.finalize()
    return nc


# ======================================================================
# host side


def _fold(W, A, Bm, bf16):
    """fq(W).T + A @ B as a single bf16 matrix [in_dim, out_dim]."""
    f32 = np.float32
    W = np.asarray(W, f32)
    s = np.maximum(np.max(np.abs(W), axis=1) / f32(127.0),
                   f32(1.0 / 127.0)).astype(f32)
    Wint = np.round(W / s[:, None]).astype(f32)
    eff = (Wint * s[:, None]).T + np.asarray(A, f32) @ np.asarray(Bm, f32)
    return eff.astype(bf16)


def make_in_maps(x, Wq, Wk, Wv, Wproj, q_gain, q_A, q_B, k_A, k_B, v_A, v_B,
                 proj_A, proj_B):
    import ml_dtypes
    bf16 = ml_dtypes.bfloat16
    f32 = np.float32

    wq_eff = _fold(Wq, q_A, q_B, bf16)          # [2048, 2048]
    wk_eff = _fold(Wk, k_A, k_B, bf16)          # [2048, 512]
    wv_eff = _fold(Wv, v_A, v_B, bf16)          # [2048, 512]
    wp_eff = _fold(Wproj, proj_A, proj_B, bf16)  # [2048, 2048]

    inv_freq = (f32(1.0) / (f32(RBASE) ** (np.arange(0, ROPE_DIMS, 2,
                dtype=f32) / f32(ROPE_DIMS)))).astype(f32)
    t = np.arange(S, dtype=f32)
    freqs = np.outer(t, inv_freq).astype(f32)
    cos = np.cos(freqs).astype(f32)
    sin = np.sin(freqs).astype(f32)
    q_gain = np.asarray(q_gain, f32)
    x = np.asarray(x, f32)

    in_maps = []
    for c in range(8):
        b, g = divmod(c, 4)
        fq0, fq1 = 512 * g, 512 * (g + 1)
        fk0, fk1 = 128 * g, 128 * (g + 1)
        in_maps.append({
            "xT": np.ascontiguousarray(x[b].T).astype(bf16),
            "wq": np.ascontiguousarray(wq_eff[:, fq0:fq1]),
            "wkv": np.ascontiguousarray(
                np.concatenate([wk_eff[:, fk0:fk1], wv_eff[:, fk0:fk1]],
                               axis=1)),
            "wp": np.ascontiguousarray(wp_eff[fq0:fq1, :]),
            "cs": cos,
            "sn": sin,
            "gn": np.ascontiguousarray(q_gain[None, 4 * g:4 * (g + 1)]),
        })
    return in_maps


def assemble(res):
    out = np.empty((B, S, DIM), np.float32)
    for b in range(B):
        acc = res.results[4 * b]["outT"].astype(np.float32)
        for g in range(1, 4):
            acc = acc + res.results[4 * b + g]["outT"].astype(np.float32)
        out[b] = acc.T
    return out


_PROGRAM = None


def kernel(**inputs):
    global _PROGRAM
    if _PROGRAM is None:
        _PROGRAM = build_program()
    in_maps = make_in_maps(**inputs)
    res = run_bass_kernel_spmd(_PROGRAM, in_maps, core_ids=list(range(8)))
    return assemble(res)
